# revision 9
# baseline (speedup 1.0000x reference)
"""DeepGCN (gnn_message_passing) Trainium2 Bass kernel, 8-way node-sharded SPMD.

Strategy (per core, nodes sharded 8 ways):
- Activations kept transposed hT [128 feats, RPAD rows] in SBUF.
- Dense y = h@W + b: PE matmuls lhsT=hT-tile rhs=W (+rank-1 ones-matmul bias)
  -> row-major y tiles -> Act-engine evict (f32->f16) -> DMA to DRAM
  ag_in[l][h] [HALF, F] (rows split at HALF) -> one AllGather PER HALF ->
  tables[l][h] [TBL=HALF*8, F] (Shared).  Table row indices stay < 32768
  so gather indices fit int16.
- spmm out[r] = sum_e val[e] * y[col[e]] split into two passes by source
  half: pass A (half 0) accumulates per-block PSUM and parks partial sums
  in an SBUF acc buffer (Act copy); pass B (half 1) finishes, combines
  (Pool add), applies relu/residual (Act), updates hT (DVE add).  The
  half-1 AllGather overlaps pass A; next layer's dense is fused per-block
  into pass B so the next half-0 AllGather launches mid-pass.
- Selector SEL[e,r] = (r==rowrel[e])*val[e] built batched per (superblock,
  half): two DVE tensor_tensor ops over [128e, 128r, NPT] with all last
  dims packed (materialized iota_rep constant) -> 2x fp16 DVE mode.
- Gathers: one dma_gather per (superblock, half) (single_packet=False) to
  amortize the per-call SWDGE fixed overhead; elem = 256 B (128 x f16).

Edges preprocessed on host (numpy): sorted by destination row-block, split
per block into the two table halves, padded to a fixed number of 128-edge
tiles per (block, half) so one static program serves all 8 cores.
"""

import numpy as np

import concourse.bacc as bacc
import concourse.bass as bass
import concourse.mybir as mybir
import concourse.tile as tile
from concourse import library_config
from concourse.bass_utils import run_bass_kernel_spmd

NCORES = 8
P = 128


class Cfg:
    def __init__(self, N=40000, E=640000, DIN=256, H=128, C=64, L=2, SBB=5,
                 big_gather=True):
        assert N % NCORES == 0
        self.N, self.E, self.DIN, self.H, self.C, self.L = N, E, DIN, H, C, L
        self.NSH = N // NCORES                    # rows per core
        self.NBLK = -(-self.NSH // P)             # 128-row blocks per core
        self.RPAD = self.NBLK * P
        assert self.NSH % 2 == 0
        self.HALF = self.NSH // 2                 # rows per table half per core
        self.TBL = self.HALF * NCORES             # rows per gather table
        assert self.TBL < 32768, "gather indices must fit int16"
        assert self.NBLK % SBB == 0
        self.SBB = SBB                            # blocks per superblock
        self.NSB = self.NBLK // SBB
        self.big_gather = big_gather
        self.tdt = mybir.dt.float16
        self.tnp = np.float16


CFG_FULL = Cfg()


# ---------------------------------------------------------------- host side


def _pack_idx(idx_flat):
    """[n] int16 -> [128, n//16]: slot i -> partition i%16, col i//16, x8 replicated."""
    n = idx_flat.shape[-1]
    t = idx_flat.reshape(*idx_flat.shape[:-1], n // 16, 16)
    t = np.swapaxes(t, -1, -2)                    # [..., 16, n//16]
    return np.tile(t, (1,) * (t.ndim - 2) + (8, 1)).astype(np.int16)


def _pack_pt(a_flat):
    """[n] -> [128, n//128]: slot i -> [i%128, i//128]."""
    n = a_flat.shape[-1]
    t = a_flat.reshape(*a_flat.shape[:-1], n // 128, 128)
    return np.swapaxes(t, -1, -2).copy()


def preprocess(cfg, x, edge_row, edge_col, edge_val):
    """Shard x, build per-core gather/selector metadata. Returns (per_core, TPB)."""
    er = np.asarray(edge_row).astype(np.int64)
    ec = np.asarray(edge_col).astype(np.int64)
    ev = np.asarray(edge_val).astype(np.float32)

    owner = er // cfg.NSH
    row_loc = er % cfg.NSH
    blk = row_loc // P                            # block within core
    rel = (row_loc % P).astype(np.float32)
    c_owner = ec // cfg.NSH
    c_loc = ec % cfg.NSH
    half = (c_loc >= cfg.HALF).astype(np.int64)
    tbl_idx = (c_owner * cfg.HALF + c_loc - half * cfg.HALF).astype(np.int64)

    cores = []
    max_cnt = 0
    for r in range(NCORES):
        m = owner == r
        cores.append((blk[m], half[m], tbl_idx[m], ev[m], rel[m]))
        key = blk[m] * 2 + half[m]
        cnt = np.bincount(key, minlength=cfg.NBLK * 2)
        max_cnt = max(max_cnt, int(cnt.max()))
    TPB = max(1, -(-max_cnt // P))                # tiles per (block, half)
    NIDX = cfg.SBB * TPB * P                      # gather-call size

    per_core = []
    for r in range(NCORES):
        b, h, ti, v, rl = cores[r]
        key = b * 2 + h
        # secondary sort by table index: monotone gather addresses within each
        # (block, half) group give far better HBM row locality
        order = np.argsort(key * 32768 + ti, kind="stable")
        b, h, ti, v, rl = b[order], h[order], ti[order], v[order], rl[order]
        cnt = np.bincount(key[order], minlength=cfg.NBLK * 2)
        # slot of edge j within its (b,h) group
        within = np.arange(len(b)) - np.repeat(
            np.concatenate([[0], np.cumsum(cnt)[:-1]]), cnt)
        # flat slot in [h, s, NIDX] layout
        s = b // cfg.SBB
        bb = b % cfg.SBB
        slot = bb * TPB * P + within
        idx_arr = np.zeros((2, cfg.NSB, NIDX), np.int16)
        val_arr = np.zeros((2, cfg.NSB, NIDX), np.float32)
        row_arr = np.zeros((2, cfg.NSB, NIDX), np.float32)
        idx_arr[h, s, slot] = ti.astype(np.int16)
        val_arr[h, s, slot] = v
        row_arr[h, s, slot] = rl

        xT = np.zeros((cfg.DIN, cfg.RPAD), np.float32)
        xT[:, : cfg.NSH] = np.asarray(x[r * cfg.NSH:(r + 1) * cfg.NSH]).T
        per_core.append(dict(
            xT=np.ascontiguousarray(xT),
            idx=_pack_idx(idx_arr),                       # [2,NSB,128,NIDX//16]
            val=_pack_pt(val_arr).astype(cfg.tnp),        # [2,NSB,128,SBB*TPB]
            rowrel=_pack_pt(row_arr).astype(cfg.tnp),
        ))
    return per_core, TPB


# -------------------------------------------------------------- device side


def build_program(cfg, TPB, dt_val, no_cc=False):
    H, C, DIN, L = cfg.H, cfg.C, cfg.DIN, cfg.L
    NIDX = cfg.SBB * TPB * P
    NPT = cfg.SBB * TPB
    NL = L + 2                                    # number of tables/layers
    # final layer zero-padded C->H so gather elems stay 256 B (fp16 x 128)
    fdims = [H] * (L + 1) + [H]
    fdt = cfg.tdt
    f32 = mybir.dt.float32
    relu = mybir.ActivationFunctionType.Relu
    fcopy = mybir.ActivationFunctionType.Copy

    nc = bacc.Bacc("TRN2", target_bir_lowering=False, debug=False,
                   num_devices=NCORES)

    xT_d = nc.dram_tensor("xT", [DIN, cfg.RPAD], f32, kind="ExternalInput")
    w1_d = nc.dram_tensor("w1", [DIN, H], f32, kind="ExternalInput")
    b1_d = nc.dram_tensor("b1", [1, H], f32, kind="ExternalInput")
    wm_d = nc.dram_tensor("wm", [L, H, H], f32, kind="ExternalInput")
    bm_d = nc.dram_tensor("bm", [L, 1, H], f32, kind="ExternalInput")
    w2_d = nc.dram_tensor("w2", [H, H], f32, kind="ExternalInput")
    b2_d = nc.dram_tensor("b2", [1, H], f32, kind="ExternalInput")
    iota_d = nc.dram_tensor("iota_rep", [P, P, NPT], fdt, kind="ExternalInput")
    idx_d = nc.dram_tensor("idx", [2, cfg.NSB, P, NIDX // 16], mybir.dt.int16,
                           kind="ExternalInput")
    val_d = nc.dram_tensor("val", [2, cfg.NSB, P, NPT], fdt,
                           kind="ExternalInput")
    row_d = nc.dram_tensor("rowrel", [2, cfg.NSB, P, NPT], fdt,
                           kind="ExternalInput")
    out_d = nc.dram_tensor("out", [cfg.NSH, C], f32, kind="ExternalOutput")

    ag_in = [[nc.dram_tensor(f"ag{l}_{h}", [cfg.HALF, fdims[l]], fdt)
              for h in (0, 1)] for l in range(NL)]
    tables = [[nc.dram_tensor(f"table{l}_{h}", [cfg.TBL, fdims[l]], fdt,
                              addr_space="Shared")
               for h in (0, 1)] for l in range(NL)]

    with tile.TileContext(nc) as tc:
        import contextlib
        with contextlib.ExitStack() as ctx:
            const = ctx.enter_context(tc.tile_pool(name="const", bufs=1))
            htp = ctx.enter_context(tc.tile_pool(name="ht", bufs=1))
            accp = ctx.enter_context(tc.tile_pool(name="acc", bufs=1))
            psum = ctx.enter_context(tc.tile_pool(name="psum", bufs=5, space="PSUM"))
            psumd = ctx.enter_context(tc.tile_pool(name="psumd", bufs=3, space="PSUM"))
            meta = ctx.enter_context(tc.tile_pool(name="meta", bufs=3))
            gpool = ctx.enter_context(tc.tile_pool(name="g", bufs=2))
            selp = ctx.enter_context(tc.tile_pool(name="sel", bufs=2))
            yp = ctx.enter_context(tc.tile_pool(name="y", bufs=4))

            nc.gpsimd.load_library(library_config.mlp)

            # ---- constants
            nkt = DIN // P                       # k-tiles for layer-1 dense
            w1_sb = [const.tile([P, H], f32, name=f"w1sb{k}")
                     for k in range(nkt)]
            for k in range(nkt):
                nc.sync.dma_start(w1_sb[k][:], w1_d[k * P:(k + 1) * P, :])
            b1_sb = const.tile([1, H], f32)
            nc.sync.dma_start(b1_sb[:], b1_d[:])
            wm_sb = [const.tile([P, H], f32, name=f"wmsb{i}")
                     for i in range(L)]
            bm_sb = [const.tile([1, H], f32, name=f"bmsb{i}")
                     for i in range(L)]
            for i in range(L):
                nc.sync.dma_start(wm_sb[i][:], wm_d[i])
                nc.sync.dma_start(bm_sb[i][:], bm_d[i])
            w2_sb = const.tile([P, H], f32)
            nc.sync.dma_start(w2_sb[:], w2_d[:])
            b2_sb = const.tile([1, H], f32)
            nc.sync.dma_start(b2_sb[:], b2_d[:])
            iota_sb = const.tile([P, P, NPT], fdt)
            nc.sync.dma_start(iota_sb[:], iota_d[:])
            ones_sb = const.tile([1, P], f32)
            nc.vector.memset(ones_sb[:], 1.0)

            ht = htp.tile([P, cfg.RPAD], f32)
            acc = accp.tile([P, cfg.RPAD], f32)

            dense_w = [w1_sb] + [[wm_sb[i]] for i in range(L)] + [[w2_sb]]
            dense_b = [b1_sb] + [bm_sb[i] for i in range(L)] + [b2_sb]

            def dense_block(l, m, lhs_tiles):
                """y[block m] = lhsT.T @ W + b -> ag_in[l] halves (f16)."""
                F = fdims[l]
                ps = psumd.tile([P, F], f32, tag="psd")
                for k, lt in enumerate(lhs_tiles):
                    nc.tensor.matmul(
                        out=ps[:], lhsT=lt[:, m * P:(m + 1) * P],
                        rhs=dense_w[l][k][:], start=(k == 0), stop=False)
                nc.tensor.matmul(out=ps[:], lhsT=ones_sb[:], rhs=dense_b[l][:],
                                 start=False, stop=True)
                ysb = yp.tile([P, F], fdt, tag="ysb")
                nc.scalar.activation(out=ysb[:], in_=ps[:], func=fcopy)
                r0 = m * P
                r1 = min(cfg.NSH, r0 + P)
                for h in (0, 1):
                    lo = max(r0, h * cfg.HALF)
                    hi = min(r1, (h + 1) * cfg.HALF)
                    if lo < hi:
                        nc.sync.dma_start(
                            out=ag_in[l][h][lo - h * cfg.HALF:hi - h * cfg.HALF, :],
                            in_=ysb[lo - r0:hi - r0, :])

            def allgather(l, h):
                if no_cc:
                    # timeline-profiling stand-in: local copy, same deps
                    nc.sync.dma_start(out=tables[l][h][0:cfg.HALF, :],
                                      in_=ag_in[l][h][:])
                    return
                nc.gpsimd.collective_compute(
                    "AllGather", mybir.AluOpType.bypass,
                    replica_groups=[list(range(NCORES))],
                    ins=[ag_in[l][h][:]], outs=[tables[l][h][:]])

            def spmm_pass(l, h, fuse_dense):
                """One half-pass of spmm over tables[l][h].

                h==0: park per-block partial sums in acc.
                h==1: combine with acc, apply layer update, write ht/out;
                      fuse_dense: emit next layer's dense per finished block;
                      returns after issuing AG(l+1) chunks at s==NSB//2-1 / end.
                """
                final = l == NL - 1
                F = fdims[l]
                for s in range(cfg.NSB):
                    it = meta.tile([P, NIDX // 16], mybir.dt.int16, tag="it")
                    nc.sync.dma_start(it[:], idx_d[h, s])
                    vt = meta.tile([P, NPT], fdt, tag="vt")
                    nc.sync.dma_start(vt[:], val_d[h, s])
                    rt = meta.tile([P, NPT], fdt, tag="rt")
                    nc.sync.dma_start(rt[:], row_d[h, s])
                    gt = gpool.tile([P, NPT, F], fdt, tag="g")
                    if cfg.big_gather:
                        nc.gpsimd.dma_gather(
                            gt[:], tables[l][h][:], it[:], NIDX, NIDX, F,
                            single_packet=False)
                    else:
                        CH = 7   # <=56 descs/lane for single_packet coalescing
                        for c0 in range(0, NPT, CH):
                            c1 = min(NPT, c0 + CH)
                            nc.gpsimd.dma_gather(
                                gt[:, c0:c1, :], tables[l][h][:],
                                it[:, c0 * 8:c1 * 8],
                                (c1 - c0) * P, (c1 - c0) * P, F,
                                single_packet=True)
                    # batched selector: SEL[e, r, j] = (r == rowrel[e,j]) * val[e,j]
                    # all operands' last dims packed -> 2x fp16 DVE mode
                    sel = selp.tile([P, P, NPT], fdt, tag="sel")
                    nc.vector.tensor_tensor(
                        out=sel[:], in0=iota_sb[:],
                        in1=rt[:][:, None, :].to_broadcast([P, P, NPT]),
                        op=mybir.AluOpType.is_equal)
                    nc.vector.tensor_tensor(
                        out=sel[:], in0=sel[:],
                        in1=vt[:][:, None, :].to_broadcast([P, P, NPT]),
                        op=mybir.AluOpType.mult)
                    for bb in range(cfg.SBB):
                        b = s * cfg.SBB + bb
                        if final:
                            ps = psum.tile([P, F], f32, tag="ps")
                        else:
                            ps = psum.tile([F, P], f32, tag="ps")
                        for t in range(TPB):
                            j = bb * TPB + t
                            if final:
                                nc.tensor.matmul(
                                    out=ps[:], lhsT=sel[:, :, j], rhs=gt[:, j, :],
                                    start=(t == 0), stop=(t == TPB - 1))
                            else:
                                nc.tensor.matmul(
                                    out=ps[:], lhsT=gt[:, j, :], rhs=sel[:, :, j],
                                    start=(t == 0), stop=(t == TPB - 1))
                        co = b * P
                        if h == 0:
                            nc.scalar.activation(out=acc[:, co:co + P],
                                                 in_=ps[:], func=fcopy)
                            continue
                        # ---- h == 1: combine + layer update
                        # (gpsimd has no PSUM port: PSUM-reading add on DVE,
                        #  relu on Act, SBUF-only residual add on gpsimd)
                        if final:
                            osb = yp.tile([P, P], f32, tag="osb")
                            nc.vector.tensor_tensor(
                                out=osb[:], in0=ps[:], in1=acc[:, co:co + P],
                                op=mybir.AluOpType.add)
                            r0 = b * P
                            r1 = min(cfg.NSH, r0 + P)
                            if r0 < r1:
                                nc.sync.dma_start(out=out_d[r0:r1, :],
                                                  in_=osb[: r1 - r0, 0:C])
                        elif l == 0:
                            tmp = yp.tile([P, P], f32, tag="tmp")
                            nc.vector.tensor_tensor(
                                out=tmp[:], in0=ps[:], in1=acc[:, co:co + P],
                                op=mybir.AluOpType.add)
                            nc.scalar.activation(
                                out=ht[:, co:co + P], in_=tmp[:], func=relu)
                        else:
                            tmp = yp.tile([P, P], f32, tag="tmp")
                            nc.vector.tensor_tensor(
                                out=tmp[:], in0=ps[:], in1=acc[:, co:co + P],
                                op=mybir.AluOpType.add)
                            u = yp.tile([P, P], f32, tag="u")
                            nc.scalar.activation(
                                out=u[:], in_=tmp[:], func=relu, scale=dt_val)
                            nc.gpsimd.tensor_add(
                                out=ht[:, co:co + P],
                                in0=ht[:, co:co + P], in1=u[:])
                        if fuse_dense:
                            dense_block(l + 1, b, [ht])
                    if h == 1 and fuse_dense and s == cfg.NSB // 2 - 1:
                        allgather(l + 1, 0)
                if h == 1 and fuse_dense:
                    allgather(l + 1, 1)

            # ---- layer 0 dense (from xT in DRAM)
            with tc.tile_pool(name="xt", bufs=1) as xtp:
                xt_sb = xtp.tile([P, nkt * cfg.RPAD], f32)
                for k in range(nkt):
                    nc.sync.dma_start(
                        xt_sb[:, k * cfg.RPAD:(k + 1) * cfg.RPAD],
                        xT_d[k * P:(k + 1) * P, :])
                xt_tiles = [xt_sb[:, k * cfg.RPAD:(k + 1) * cfg.RPAD]
                            for k in range(nkt)]
                for m in range(cfg.NBLK):
                    dense_block(0, m, xt_tiles)
                    if m == cfg.NBLK // 2 - 1:
                        allgather(0, 0)
                allgather(0, 1)
            for l in range(NL):
                spmm_pass(l, 0, fuse_dense=False)
                spmm_pass(l, 1, fuse_dense=(l < NL - 1))

    nc.compile()
    return nc


# ------------------------------------------------------------------ driver

_CACHE = {}


def _get_program(cfg, TPB, dt_val):
    key = (cfg.N, cfg.E, cfg.big_gather, TPB, float(dt_val))
    if key not in _CACHE:
        _CACHE[key] = build_program(cfg, TPB, dt_val)
    return _CACHE[key]


def prepare(cfg, inputs):
    """Preprocess inputs and build (cached) program. Returns (nc, in_maps)."""
    x = np.asarray(inputs["x"], np.float32)
    per_core, TPB = preprocess(cfg, x, inputs["edge_row"], inputs["edge_col"],
                               inputs["edge_val"])
    dt_val = float(np.asarray(inputs["time_step"]))
    nc = _get_program(cfg, TPB, dt_val)

    NPT = cfg.SBB * TPB
    iota_rep = np.broadcast_to(
        np.arange(P, dtype=cfg.tnp)[None, :, None], (P, P, NPT)).copy()
    shared = dict(
        w1=np.asarray(inputs["w1"], np.float32),
        b1=np.asarray(inputs["b1"], np.float32).reshape(1, cfg.H),
        wm=np.asarray(inputs["wm"], np.float32),
        bm=np.asarray(inputs["bm"], np.float32).reshape(cfg.L, 1, cfg.H),
        w2=np.pad(np.asarray(inputs["w2"], np.float32),
                  ((0, 0), (0, cfg.H - cfg.C))),
        b2=np.pad(np.asarray(inputs["b2"], np.float32).reshape(1, cfg.C),
                  ((0, 0), (0, cfg.H - cfg.C))),
        iota_rep=iota_rep,
    )
    in_maps = [{**shared, **pc} for pc in per_core]
    return nc, in_maps


def run(cfg, inputs):
    nc, in_maps = prepare(cfg, inputs)
    res = run_bass_kernel_spmd(nc, in_maps, list(range(NCORES)))
    out = np.concatenate([res.results[r]["out"] for r in range(NCORES)], axis=0)
    return out.astype(np.float32)


def kernel(**inputs) -> np.ndarray:
    return run(CFG_FULL, inputs)


# ---------------------------------------------------- timing helper (test use)


def make_timed_runner(nc, in_maps):
    """Build a reusable jitted runner (no donation, device-resident operands).

    Mirrors bass2jax.run_bass_via_pjrt's multi-core path but keeps the jitted
    callable and device arrays so repeated calls measure dispatch+exec only.
    Returns (call_fn, out_unpack_fn).
    """
    import jax
    from jax.sharding import Mesh, PartitionSpec
    from jax.experimental.shard_map import shard_map
    from concourse import bass2jax
    from concourse.bass2jax import _bass_exec_p, partition_id_tensor

    bass2jax.install_neuronx_cc_hook()
    n_cores = len(in_maps)
    partition_name = nc.partition_id_tensor.name if nc.partition_id_tensor else None
    in_names, out_names, out_avals, zero_outs = [], [], [], []
    for alloc in nc.m.functions[0].allocations:
        if not isinstance(alloc, mybir.MemoryLocationSet):
            continue
        name = alloc.memorylocations[0].name
        if alloc.kind == "ExternalInput":
            if name != partition_name:
                in_names.append(name)
        elif alloc.kind == "ExternalOutput":
            out_names.append(name)
            out_avals.append(jax.core.ShapedArray(
                tuple(alloc.tensor_shape), mybir.dt.np(alloc.dtype)))
            zero_outs.append(np.zeros(tuple(alloc.tensor_shape),
                                      mybir.dt.np(alloc.dtype)))
    n_params = len(in_names)
    all_names = in_names + out_names
    if partition_name is not None:
        all_names.append(partition_name)

    def _body(*args):
        operands = list(args)
        if partition_name is not None:
            operands.append(partition_id_tensor())
        return tuple(_bass_exec_p.bind(
            *operands,
            out_avals=tuple(out_avals),
            in_names=tuple(all_names),
            out_names=tuple(out_names),
            lowering_input_output_aliases=(),
            sim_require_finite=True,
            sim_require_nnan=True,
            nc=nc,
        ))

    devices = jax.devices()[:n_cores]
    mesh = Mesh(np.asarray(devices), ("core",))
    spec_in = (PartitionSpec("core"),) * (n_params + len(out_names))
    spec_out = (PartitionSpec("core"),) * len(out_names)
    fn = jax.jit(shard_map(_body, mesh=mesh, in_specs=spec_in,
                           out_specs=spec_out, check_rep=False),
                 keep_unused=True)

    sharding = jax.sharding.NamedSharding(mesh, PartitionSpec("core"))
    dev_args = []
    for i, name in enumerate(in_names):
        cat = np.concatenate([np.asarray(m[name]) for m in in_maps], axis=0)
        dev_args.append(jax.device_put(cat, sharding))
    for z in zero_outs:
        cat = np.zeros((n_cores * z.shape[0], *z.shape[1:]), z.dtype)
        dev_args.append(jax.device_put(cat, sharding))

    def call():
        outs = fn(*dev_args)
        jax.block_until_ready(outs)
        return outs

    def unpack(outs):
        return [
            {name: np.asarray(outs[i]).reshape(n_cores, *out_avals[i].shape)[c]
             for i, name in enumerate(out_names)}
            for c in range(n_cores)
        ]

    return call, unpack


# revision 18
# speedup vs baseline: 1.3232x; 1.3232x over previous
"""DeepGCN (gnn_message_passing) Trainium2 Bass kernel, 8-way node-sharded SPMD.

Strategy (per core, nodes sharded 8 ways):
- Activations kept transposed hT [128 feats, RPAD rows] in SBUF.
- Dense y = h@W + b: PE matmuls lhsT=hT-tile rhs=W (+rank-1 ones-matmul bias)
  -> row-major y tiles -> Act-engine evict (f32->f16) -> DMA to DRAM
  ag_in[l][h] [HALF, F] (rows split at HALF) -> one AllGather PER HALF ->
  tables[l][h] [TBL=HALF*8, F] (Shared).  Table row indices stay < 32768
  so gather indices fit int16.
- spmm out[r] = sum_e val[e] * y[col[e]] split into two passes by source
  half: pass A (half 0) accumulates per-block PSUM and parks partial sums
  in an SBUF acc buffer (Act copy); pass B (half 1) finishes, combines
  (Pool add), applies relu/residual (Act), updates hT (DVE add).  The
  half-1 AllGather overlaps pass A; next layer's dense is fused per-block
  into pass B so the next half-0 AllGather launches mid-pass.
- Selector SEL[e,r] = (r==rowrel[e])*val[e] built batched per (superblock,
  half): two DVE tensor_tensor ops over [128e, 128r, NPT] with all last
  dims packed (materialized iota_rep constant) -> 2x fp16 DVE mode.
- Gathers: one dma_gather per (superblock, half) (single_packet=False) to
  amortize the per-call SWDGE fixed overhead; elem = 256 B (128 x f16).

Edges preprocessed on host (numpy): sorted by destination row-block, split
per block into the two table halves, padded to a fixed number of 128-edge
tiles per (block, half) so one static program serves all 8 cores.
"""

import numpy as np

import concourse.bacc as bacc
import concourse.bass as bass
import concourse.mybir as mybir
import concourse.tile as tile
from concourse import library_config
from concourse.bass_utils import run_bass_kernel_spmd

NCORES = 8
P = 128


class Cfg:
    def __init__(self, N=40000, E=640000, DIN=256, H=128, C=64, L=2, SBB=5,
                 big_gather=False, mm_dummy=False, act_ops=True, sel_pair=True,
                 ag_point=99):
        assert N % NCORES == 0
        self.N, self.E, self.DIN, self.H, self.C, self.L = N, E, DIN, H, C, L
        self.NSH = N // NCORES                    # rows per core
        self.NBLK = -(-self.NSH // P)             # 128-row blocks per core
        self.RPAD = self.NBLK * P
        assert self.NSH % 2 == 0
        self.HALF = self.NSH // 2                 # rows per table half per core
        self.TBL = self.HALF * NCORES             # rows per gather table
        assert self.TBL < 32768, "gather indices must fit int16"
        assert self.NBLK % SBB == 0
        self.SBB = SBB                            # blocks per superblock
        self.NSB = self.NBLK // SBB
        self.big_gather = big_gather
        self.mm_dummy = mm_dummy      # timing bisect: contiguous dummy sel rhs
        self.act_ops = act_ops        # PSUM evictions on Act engine (else DVE)
        self.sel_pair = sel_pair      # selector [P,NPH,P,2] (stride-2 matmul)
        self.ag_point = ag_point      # superblock index after which AG(l+1,0) issues
        self.tdt = mybir.dt.float16
        self.tnp = np.float16


CFG_FULL = Cfg()


# ---------------------------------------------------------------- host side


def _pack_idx(idx_flat):
    """[n] int16 -> [128, n//16]: slot i -> partition i%16, col i//16, x8 replicated."""
    n = idx_flat.shape[-1]
    t = idx_flat.reshape(*idx_flat.shape[:-1], n // 16, 16)
    t = np.swapaxes(t, -1, -2)                    # [..., 16, n//16]
    return np.tile(t, (1,) * (t.ndim - 2) + (8, 1)).astype(np.int16)


def _pack_pt(a_flat):
    """[n] -> [128, n//128]: slot i -> [i%128, i//128]."""
    n = a_flat.shape[-1]
    t = a_flat.reshape(*a_flat.shape[:-1], n // 128, 128)
    return np.swapaxes(t, -1, -2).copy()


def preprocess(cfg, x, edge_row, edge_col, edge_val):
    """Shard x, build per-core gather/selector metadata. Returns (per_core, TPB)."""
    er = np.asarray(edge_row).astype(np.int64)
    ec = np.asarray(edge_col).astype(np.int64)
    ev = np.asarray(edge_val).astype(np.float32)

    owner = er // cfg.NSH
    row_loc = er % cfg.NSH
    blk = row_loc // P                            # block within core
    rel = (row_loc % P).astype(np.float32)
    c_owner = ec // cfg.NSH
    c_loc = ec % cfg.NSH
    half = (c_loc >= cfg.HALF).astype(np.int64)
    tbl_idx = (c_owner * cfg.HALF + c_loc - half * cfg.HALF).astype(np.int64)

    cores = []
    max_cnt = 0
    for r in range(NCORES):
        m = owner == r
        cores.append((blk[m], half[m], tbl_idx[m], ev[m], rel[m]))
        key = blk[m] * 2 + half[m]
        cnt = np.bincount(key, minlength=cfg.NBLK * 2)
        max_cnt = max(max_cnt, int(cnt.max()))
    TPB = max(1, -(-max_cnt // P))                # tiles per (block, half)
    NIDX = cfg.SBB * TPB * P                      # gather-call size

    per_core = []
    for r in range(NCORES):
        b, h, ti, v, rl = cores[r]
        key = b * 2 + h
        # secondary sort by table index: monotone gather addresses within each
        # (block, half) group give far better HBM row locality
        order = np.argsort(key * 32768 + ti, kind="stable")
        b, h, ti, v, rl = b[order], h[order], ti[order], v[order], rl[order]
        cnt = np.bincount(key[order], minlength=cfg.NBLK * 2)
        # slot of edge j within its (b,h) group
        within = np.arange(len(b)) - np.repeat(
            np.concatenate([[0], np.cumsum(cnt)[:-1]]), cnt)
        # flat slot in [h, s, NIDX] layout
        s = b // cfg.SBB
        bb = b % cfg.SBB
        slot = bb * TPB * P + within
        idx_arr = np.zeros((2, cfg.NSB, NIDX), np.int16)
        val_arr = np.zeros((2, cfg.NSB, NIDX), np.float32)
        row_arr = np.zeros((2, cfg.NSB, NIDX), np.float32)
        idx_arr[h, s, slot] = ti.astype(np.int16)
        val_arr[h, s, slot] = v
        row_arr[h, s, slot] = rl

        xT = np.zeros((cfg.DIN, cfg.RPAD), np.float32)
        xT[:, : cfg.NSH] = np.asarray(x[r * cfg.NSH:(r + 1) * cfg.NSH]).T
        NPT = cfg.SBB * TPB
        val_p = _pack_pt(val_arr).astype(cfg.tnp)         # [2,NSB,128,NPT]
        row_p = _pack_pt(row_arr).astype(cfg.tnp)
        if cfg.sel_pair:
            NPT_P = NPT + NPT % 2
            pad = ((0, 0), (0, 0), (0, 0), (0, NPT_P - NPT))
            val_p = np.pad(val_p, pad).reshape(2, cfg.NSB, P, NPT_P // 2, 2)
            row_p = np.pad(row_p, pad).reshape(2, cfg.NSB, P, NPT_P // 2, 2)
        per_core.append(dict(
            xT=np.ascontiguousarray(xT),
            idx=_pack_idx(idx_arr),                       # [2,NSB,128,NIDX//16]
            val=val_p,
            rowrel=row_p,
        ))
    return per_core, TPB


# -------------------------------------------------------------- device side


def build_program(cfg, TPB, dt_val, no_cc=False):
    H, C, DIN, L = cfg.H, cfg.C, cfg.DIN, cfg.L
    NIDX = cfg.SBB * TPB * P
    NPT = cfg.SBB * TPB
    NL = L + 2                                    # number of tables/layers
    # final layer zero-padded C->H so gather elems stay 256 B (fp16 x 128)
    fdims = [H] * (L + 1) + [H]
    fdt = cfg.tdt
    f32 = mybir.dt.float32
    relu = mybir.ActivationFunctionType.Relu
    fcopy = mybir.ActivationFunctionType.Copy

    nc = bacc.Bacc("TRN2", target_bir_lowering=False, debug=False,
                   num_devices=NCORES)

    xT_d = nc.dram_tensor("xT", [DIN, cfg.RPAD], f32, kind="ExternalInput")
    w1_d = nc.dram_tensor("w1", [DIN, H], f32, kind="ExternalInput")
    b1_d = nc.dram_tensor("b1", [1, H], f32, kind="ExternalInput")
    wm_d = nc.dram_tensor("wm", [L, H, H], f32, kind="ExternalInput")
    bm_d = nc.dram_tensor("bm", [L, 1, H], f32, kind="ExternalInput")
    w2_d = nc.dram_tensor("w2", [H, H], f32, kind="ExternalInput")
    b2_d = nc.dram_tensor("b2", [1, H], f32, kind="ExternalInput")
    NPT_P = NPT + NPT % 2                         # padded even for pair layout
    NPH = NPT_P // 2
    iota_shape = [P, NPH, P, 2] if cfg.sel_pair else [P, P, NPT]
    iota_d = nc.dram_tensor("iota_rep", iota_shape, fdt, kind="ExternalInput")
    idx_d = nc.dram_tensor("idx", [2, cfg.NSB, P, NIDX // 16], mybir.dt.int16,
                           kind="ExternalInput")
    meta_shape = [2, cfg.NSB, P, NPH, 2] if cfg.sel_pair \
        else [2, cfg.NSB, P, NPT]
    val_d = nc.dram_tensor("val", meta_shape, fdt, kind="ExternalInput")
    row_d = nc.dram_tensor("rowrel", meta_shape, fdt, kind="ExternalInput")
    out_d = nc.dram_tensor("out", [cfg.NSH, C], f32, kind="ExternalOutput")

    ag_in = [[nc.dram_tensor(f"ag{l}_{h}", [cfg.HALF, fdims[l]], fdt)
              for h in (0, 1)] for l in range(NL)]
    tables = [[nc.dram_tensor(f"table{l}_{h}", [cfg.TBL, fdims[l]], fdt,
                              addr_space="Shared")
               for h in (0, 1)] for l in range(NL)]

    with tile.TileContext(nc) as tc:
        import contextlib
        with contextlib.ExitStack() as ctx:
            const = ctx.enter_context(tc.tile_pool(name="const", bufs=1))
            htp = ctx.enter_context(tc.tile_pool(name="ht", bufs=1))
            accp = ctx.enter_context(tc.tile_pool(name="acc", bufs=1))
            psum = ctx.enter_context(tc.tile_pool(name="psum", bufs=5, space="PSUM"))
            psumd = ctx.enter_context(tc.tile_pool(name="psumd", bufs=3, space="PSUM"))
            meta = ctx.enter_context(tc.tile_pool(name="meta", bufs=3))
            gpool = ctx.enter_context(tc.tile_pool(name="g", bufs=2))
            selp = ctx.enter_context(tc.tile_pool(name="sel", bufs=2))
            yp = ctx.enter_context(tc.tile_pool(name="y", bufs=4))

            nc.gpsimd.load_library(library_config.mlp)

            # ---- constants
            nkt = DIN // P                       # k-tiles for layer-1 dense
            w1_sb = [const.tile([P, H], f32, name=f"w1sb{k}")
                     for k in range(nkt)]
            for k in range(nkt):
                nc.sync.dma_start(w1_sb[k][:], w1_d[k * P:(k + 1) * P, :])
            b1_sb = const.tile([1, H], f32)
            nc.sync.dma_start(b1_sb[:], b1_d[:])
            wm_sb = [const.tile([P, H], f32, name=f"wmsb{i}")
                     for i in range(L)]
            bm_sb = [const.tile([1, H], f32, name=f"bmsb{i}")
                     for i in range(L)]
            for i in range(L):
                nc.sync.dma_start(wm_sb[i][:], wm_d[i])
                nc.sync.dma_start(bm_sb[i][:], bm_d[i])
            w2_sb = const.tile([P, H], f32)
            nc.sync.dma_start(w2_sb[:], w2_d[:])
            b2_sb = const.tile([1, H], f32)
            nc.sync.dma_start(b2_sb[:], b2_d[:])
            iota_sb = const.tile(iota_shape, fdt)
            nc.sync.dma_start(iota_sb[:], iota_d[:])
            ones_sb = const.tile([1, P], f32)
            nc.vector.memset(ones_sb[:], 1.0)
            if cfg.mm_dummy:
                dum_sb = const.tile([P, P], fdt)
                nc.vector.memset(dum_sb[:], 0.0)

            def evict(out_ap, in_ap):
                if cfg.act_ops:
                    nc.scalar.activation(out=out_ap, in_=in_ap, func=fcopy)
                else:
                    nc.vector.tensor_copy(out=out_ap, in_=in_ap)


            ht = htp.tile([P, cfg.RPAD], f32)
            acc = accp.tile([P, cfg.RPAD], f32)

            dense_w = [w1_sb] + [[wm_sb[i]] for i in range(L)] + [[w2_sb]]
            dense_b = [b1_sb] + [bm_sb[i] for i in range(L)] + [b2_sb]

            def dense_block(l, m, lhs_tiles):
                """y[block m] = lhsT.T @ W + b -> ag_in[l] halves (f16)."""
                F = fdims[l]
                ps = psumd.tile([P, F], f32, tag="psd")
                for k, lt in enumerate(lhs_tiles):
                    nc.tensor.matmul(
                        out=ps[:], lhsT=lt[:, m * P:(m + 1) * P],
                        rhs=dense_w[l][k][:], start=(k == 0), stop=False)
                nc.tensor.matmul(out=ps[:], lhsT=ones_sb[:], rhs=dense_b[l][:],
                                 start=False, stop=True)
                ysb = yp.tile([P, F], fdt, tag="ysb")
                evict(ysb[:], ps[:])
                r0 = m * P
                r1 = min(cfg.NSH, r0 + P)
                for h in (0, 1):
                    lo = max(r0, h * cfg.HALF)
                    hi = min(r1, (h + 1) * cfg.HALF)
                    if lo < hi:
                        nc.scalar.dma_start(
                            out=ag_in[l][h][lo - h * cfg.HALF:hi - h * cfg.HALF, :],
                            in_=ysb[lo - r0:hi - r0, :])

            def allgather(l, h):
                if no_cc:
                    # timeline-profiling stand-in: local copy, same deps
                    nc.sync.dma_start(out=tables[l][h][0:cfg.HALF, :],
                                      in_=ag_in[l][h][:])
                    return
                nc.gpsimd.collective_compute(
                    "AllGather", mybir.AluOpType.bypass,
                    replica_groups=[list(range(NCORES))],
                    ins=[ag_in[l][h][:]], outs=[tables[l][h][:]])

            def spmm_pass(l, h, fuse_dense):
                """One half-pass of spmm over tables[l][h].

                h==0: park per-block partial sums in acc.
                h==1: combine with acc, apply layer update, write ht/out;
                      fuse_dense: emit next layer's dense per finished block;
                      returns after issuing AG(l+1) chunks at s==NSB//2-1 / end.
                """
                final = l == NL - 1
                F = fdims[l]
                for s in range(cfg.NSB):
                    it = meta.tile([P, NIDX // 16], mybir.dt.int16, tag="it")
                    nc.sync.dma_start(it[:], idx_d[h, s])
                    mshape = [P, NPH, 2] if cfg.sel_pair else [P, NPT]
                    vt = meta.tile(mshape, fdt, tag="vt")
                    nc.sync.dma_start(vt[:], val_d[h, s])
                    rt = meta.tile(mshape, fdt, tag="rt")
                    nc.sync.dma_start(rt[:], row_d[h, s])
                    gt = gpool.tile([P, NPT, F], fdt, tag="g")
                    if cfg.big_gather:
                        nc.gpsimd.dma_gather(
                            gt[:], tables[l][h][:], it[:], NIDX, NIDX, F,
                            single_packet=False)
                    else:
                        CH = 7   # <=56 descs/lane for single_packet coalescing
                        for c0 in range(0, NPT, CH):
                            c1 = min(NPT, c0 + CH)
                            nc.gpsimd.dma_gather(
                                gt[:, c0:c1, :], tables[l][h][:],
                                it[:, c0 * 8:c1 * 8],
                                (c1 - c0) * P, (c1 - c0) * P, F,
                                single_packet=True)
                    # batched selector: SEL[e, r, j] = (r == rowrel[e,j]) * val[e,j]
                    # all operands' last dims packed -> 2x fp16 DVE mode
                    if cfg.sel_pair:
                        sel = selp.tile([P, NPH, P, 2], fdt, tag="sel")
                        rt_b = rt[:][:, :, None, :].to_broadcast([P, NPH, P, 2])
                        vt_b = vt[:][:, :, None, :].to_broadcast([P, NPH, P, 2])
                    else:
                        sel = selp.tile([P, P, NPT], fdt, tag="sel")
                        rt_b = rt[:][:, None, :].to_broadcast([P, P, NPT])
                        vt_b = vt[:][:, None, :].to_broadcast([P, P, NPT])
                    nc.vector.tensor_tensor(
                        out=sel[:], in0=iota_sb[:], in1=rt_b,
                        op=mybir.AluOpType.is_equal)
                    nc.vector.tensor_tensor(
                        out=sel[:], in0=sel[:], in1=vt_b,
                        op=mybir.AluOpType.mult)
                    for bb in range(cfg.SBB):
                        b = s * cfg.SBB + bb
                        if final:
                            ps = psum.tile([P, F], f32, tag="ps")
                        else:
                            ps = psum.tile([F, P], f32, tag="ps")
                        for t in range(TPB):
                            j = bb * TPB + t
                            if cfg.mm_dummy:
                                sl = dum_sb[:]
                            elif cfg.sel_pair:
                                sl = sel[:, j // 2, :, j % 2]
                            else:
                                sl = sel[:, :, j]
                            if final:
                                nc.tensor.matmul(
                                    out=ps[:], lhsT=sl, rhs=gt[:, j, :],
                                    start=(t == 0), stop=(t == TPB - 1))
                            else:
                                nc.tensor.matmul(
                                    out=ps[:], lhsT=gt[:, j, :], rhs=sl,
                                    start=(t == 0), stop=(t == TPB - 1))
                        co = b * P
                        if h == 0:
                            evict(acc[:, co:co + P], ps[:])
                            continue
                        # ---- h == 1: combine + layer update (all DVE;
                        # dt is pre-folded into wm/bm so no scale needed)
                        if final:
                            osb = yp.tile([P, P], f32, tag="osb")
                            nc.vector.tensor_tensor(
                                out=osb[:], in0=ps[:], in1=acc[:, co:co + P],
                                op=mybir.AluOpType.add)
                            r0 = b * P
                            r1 = min(cfg.NSH, r0 + P)
                            if r0 < r1:
                                nc.scalar.dma_start(out=out_d[r0:r1, :],
                                                    in_=osb[: r1 - r0, 0:C])
                        elif l == 0:
                            tmp = yp.tile([P, P], f32, tag="tmp")
                            nc.vector.tensor_tensor(
                                out=tmp[:], in0=ps[:], in1=acc[:, co:co + P],
                                op=mybir.AluOpType.add)
                            nc.vector.tensor_scalar(
                                out=ht[:, co:co + P], in0=tmp[:],
                                scalar1=0.0, scalar2=None,
                                op0=mybir.AluOpType.max)
                        else:
                            tmp = yp.tile([P, P], f32, tag="tmp")
                            nc.vector.tensor_tensor(
                                out=tmp[:], in0=ps[:], in1=acc[:, co:co + P],
                                op=mybir.AluOpType.add)
                            # ht += relu(tmp), fused: (tmp max 0) add ht
                            nc.vector.scalar_tensor_tensor(
                                out=ht[:, co:co + P], in0=tmp[:], scalar=0.0,
                                in1=ht[:, co:co + P],
                                op0=mybir.AluOpType.max,
                                op1=mybir.AluOpType.add)
                        if fuse_dense:
                            dense_block(l + 1, b, [ht])
                    if h == 1 and fuse_dense and s == cfg.ag_point:
                        allgather(l + 1, 0)
                if h == 1 and fuse_dense:
                    if cfg.ag_point >= cfg.NSB:
                        allgather(l + 1, 0)
                    allgather(l + 1, 1)

            # ---- layer 0 dense (from xT in DRAM)
            with tc.tile_pool(name="xt", bufs=1) as xtp:
                xt_sb = xtp.tile([P, nkt * cfg.RPAD], f32)
                for k in range(nkt):
                    nc.sync.dma_start(
                        xt_sb[:, k * cfg.RPAD:(k + 1) * cfg.RPAD],
                        xT_d[k * P:(k + 1) * P, :])
                xt_tiles = [xt_sb[:, k * cfg.RPAD:(k + 1) * cfg.RPAD]
                            for k in range(nkt)]
                for m in range(cfg.NBLK):
                    dense_block(0, m, xt_tiles)
                    if m == cfg.NBLK // 2 - 1 and cfg.ag_point < cfg.NSB:
                        allgather(0, 0)
                if cfg.ag_point >= cfg.NSB:
                    allgather(0, 0)
                allgather(0, 1)
            for l in range(NL):
                spmm_pass(l, 0, fuse_dense=False)
                spmm_pass(l, 1, fuse_dense=(l < NL - 1))

    nc.compile()
    return nc


# ------------------------------------------------------------------ driver

_CACHE = {}


def _get_program(cfg, TPB, dt_val):
    key = (cfg.N, cfg.E, cfg.big_gather, cfg.sel_pair, cfg.act_ops,
           cfg.mm_dummy, cfg.ag_point, TPB, float(dt_val))
    if key not in _CACHE:
        _CACHE[key] = build_program(cfg, TPB, dt_val)
    return _CACHE[key]


def make_in_maps(cfg, inputs, per_core, TPB):
    dt_val = float(np.asarray(inputs["time_step"]))
    NPT = cfg.SBB * TPB
    NPH = (NPT + NPT % 2) // 2
    ar = np.arange(P, dtype=cfg.tnp)
    if cfg.sel_pair:
        iota_rep = np.broadcast_to(
            ar[None, None, :, None], (P, NPH, P, 2)).copy()
    else:
        iota_rep = np.broadcast_to(ar[None, :, None], (P, P, NPT)).copy()
    shared = dict(
        w1=np.asarray(inputs["w1"], np.float32),
        b1=np.asarray(inputs["b1"], np.float32).reshape(1, cfg.H),
        wm=np.asarray(inputs["wm"], np.float32) * dt_val,
        bm=np.asarray(inputs["bm"], np.float32).reshape(cfg.L, 1, cfg.H)
        * dt_val,
        w2=np.pad(np.asarray(inputs["w2"], np.float32),
                  ((0, 0), (0, cfg.H - cfg.C))),
        b2=np.pad(np.asarray(inputs["b2"], np.float32).reshape(1, cfg.C),
                  ((0, 0), (0, cfg.H - cfg.C))),
        iota_rep=iota_rep,
    )
    return [{**shared, **pc} for pc in per_core]


def prepare(cfg, inputs):
    """Preprocess inputs and build (cached) program. Returns (nc, in_maps)."""
    x = np.asarray(inputs["x"], np.float32)
    per_core, TPB = preprocess(cfg, x, inputs["edge_row"], inputs["edge_col"],
                               inputs["edge_val"])
    dt_val = float(np.asarray(inputs["time_step"]))
    nc = _get_program(cfg, TPB, dt_val)
    return nc, make_in_maps(cfg, inputs, per_core, TPB)


def run(cfg, inputs):
    nc, in_maps = prepare(cfg, inputs)
    res = run_bass_kernel_spmd(nc, in_maps, list(range(NCORES)))
    out = np.concatenate([res.results[r]["out"] for r in range(NCORES)], axis=0)
    return out.astype(np.float32)


def kernel(**inputs) -> np.ndarray:
    return run(CFG_FULL, inputs)


# ---------------------------------------------------- timing helper (test use)


def make_timed_runner(nc, in_maps):
    """Build a reusable jitted runner (no donation, device-resident operands).

    Mirrors bass2jax.run_bass_via_pjrt's multi-core path but keeps the jitted
    callable and device arrays so repeated calls measure dispatch+exec only.
    Returns (call_fn, out_unpack_fn).
    """
    import jax
    from jax.sharding import Mesh, PartitionSpec
    from jax.experimental.shard_map import shard_map
    from concourse import bass2jax
    from concourse.bass2jax import _bass_exec_p, partition_id_tensor

    bass2jax.install_neuronx_cc_hook()
    n_cores = len(in_maps)
    partition_name = nc.partition_id_tensor.name if nc.partition_id_tensor else None
    in_names, out_names, out_avals, zero_outs = [], [], [], []
    for alloc in nc.m.functions[0].allocations:
        if not isinstance(alloc, mybir.MemoryLocationSet):
            continue
        name = alloc.memorylocations[0].name
        if alloc.kind == "ExternalInput":
            if name != partition_name:
                in_names.append(name)
        elif alloc.kind == "ExternalOutput":
            out_names.append(name)
            out_avals.append(jax.core.ShapedArray(
                tuple(alloc.tensor_shape), mybir.dt.np(alloc.dtype)))
            zero_outs.append(np.zeros(tuple(alloc.tensor_shape),
                                      mybir.dt.np(alloc.dtype)))
    n_params = len(in_names)
    all_names = in_names + out_names
    if partition_name is not None:
        all_names.append(partition_name)

    def _body(*args):
        operands = list(args)
        if partition_name is not None:
            operands.append(partition_id_tensor())
        return tuple(_bass_exec_p.bind(
            *operands,
            out_avals=tuple(out_avals),
            in_names=tuple(all_names),
            out_names=tuple(out_names),
            lowering_input_output_aliases=(),
            sim_require_finite=True,
            sim_require_nnan=True,
            nc=nc,
        ))

    devices = jax.devices()[:n_cores]
    mesh = Mesh(np.asarray(devices), ("core",))
    spec_in = (PartitionSpec("core"),) * (n_params + len(out_names))
    spec_out = (PartitionSpec("core"),) * len(out_names)
    fn = jax.jit(shard_map(_body, mesh=mesh, in_specs=spec_in,
                           out_specs=spec_out, check_rep=False),
                 keep_unused=True)

    sharding = jax.sharding.NamedSharding(mesh, PartitionSpec("core"))
    dev_args = []
    for i, name in enumerate(in_names):
        cat = np.concatenate([np.asarray(m[name]) for m in in_maps], axis=0)
        dev_args.append(jax.device_put(cat, sharding))
    for z in zero_outs:
        cat = np.zeros((n_cores * z.shape[0], *z.shape[1:]), z.dtype)
        dev_args.append(jax.device_put(cat, sharding))

    def call():
        outs = fn(*dev_args)
        jax.block_until_ready(outs)
        return outs

    def unpack(outs):
        return [
            {name: np.asarray(outs[i]).reshape(n_cores, *out_avals[i].shape)[c]
             for i, name in enumerate(out_names)}
            for c in range(n_cores)
        ]

    return call, unpack


# revision 25
# speedup vs baseline: 1.9732x; 1.4912x over previous
"""DeepGCN (gnn_message_passing) Trainium2 Bass kernel, 8-way node-sharded SPMD.

Strategy (per core, nodes sharded 8 ways):
- Activations kept transposed hT [128 feats, RPAD rows] in SBUF.
- Dense y = h@W + b: PE matmuls lhsT=hT-tile rhs=W (+rank-1 ones-matmul bias)
  -> row-major y tiles -> Act-engine evict (f32->f16) -> DMA to DRAM
  ag_in[l][h] [HALF, F] (rows split at HALF) -> one AllGather PER HALF ->
  tables[l][h] [TBL=HALF*8, F] (Shared).  Table row indices stay < 32768
  so gather indices fit int16.
- spmm out[r] = sum_e val[e] * y[col[e]] split into two passes by source
  half: pass A (half 0) accumulates per-block PSUM and parks partial sums
  in an SBUF acc buffer (Act copy); pass B (half 1) finishes, combines
  (Pool add), applies relu/residual (Act), updates hT (DVE add).  The
  half-1 AllGather overlaps pass A; next layer's dense is fused per-block
  into pass B so the next half-0 AllGather launches mid-pass.
- Selector SEL[e,r] = (r==rowrel[e])*val[e] built batched per (superblock,
  half): two DVE tensor_tensor ops over [128e, 128r, NPT] with all last
  dims packed (materialized iota_rep constant) -> 2x fp16 DVE mode.
- Gathers: one dma_gather per (superblock, half) (single_packet=False) to
  amortize the per-call SWDGE fixed overhead; elem = 256 B (128 x f16).

Edges preprocessed on host (numpy): sorted by destination row-block, split
per block into the two table halves, padded to a fixed number of 128-edge
tiles per (block, half) so one static program serves all 8 cores.
"""

import numpy as np

import concourse.bacc as bacc
import concourse.bass as bass
import concourse.mybir as mybir
import concourse.tile as tile
from concourse import library_config
from concourse.bass_utils import run_bass_kernel_spmd

NCORES = 8
P = 128


class Cfg:
    def __init__(self, N=40000, E=640000, DIN=256, H=128, C=64, L=2, SBB=5,
                 big_gather=False, mm_dummy=False, act_ops=True, sel_pair=True,
                 ag_point=3, gath_lite=False, sel_lite=False, mm_lite=False,
                 nq=4, ch=7, negpad=False):
        assert N % NCORES == 0
        self.N, self.E, self.DIN, self.H, self.C, self.L = N, E, DIN, H, C, L
        self.NSH = N // NCORES                    # rows per core
        self.NBLK = -(-self.NSH // P)             # 128-row blocks per core
        self.RPAD = self.NBLK * P
        assert self.NSH % 2 == 0
        self.HALF = self.NSH // 2                 # rows per table half per core
        self.TBL = self.HALF * NCORES             # rows per gather table
        assert self.TBL < 32768, "gather indices must fit int16"
        assert self.NBLK % SBB == 0
        self.SBB = SBB                            # blocks per superblock
        self.NSB = self.NBLK // SBB
        self.big_gather = big_gather
        self.mm_dummy = mm_dummy      # timing bisect: contiguous dummy sel rhs
        self.act_ops = act_ops        # PSUM evictions on Act engine (else DVE)
        self.sel_pair = sel_pair      # selector [P,NPH,P,2] (stride-2 matmul)
        self.ag_point = ag_point      # superblock index after which AG(l+1,0) issues
        self.gath_lite = gath_lite    # timing bisect: 1/6 of gather descriptors
        self.sel_lite = sel_lite      # timing bisect: skip selector build
        self.mm_lite = mm_lite        # timing bisect: 1 matmul per block
        self.nq = nq                  # SWDGE queues for gather round-robin
        self.ch = ch                  # gather chunk tiles (<=8 for single_packet)
        self.negpad = negpad          # idx=-1 padding + block-aligned chunks
        self.tdt = mybir.dt.float16
        self.tnp = np.float16


CFG_FULL = Cfg()


# ---------------------------------------------------------------- host side


def _pack_idx(idx_flat):
    """[n] int16 -> [128, n//16]: slot i -> partition i%16, col i//16, x8 replicated."""
    n = idx_flat.shape[-1]
    t = idx_flat.reshape(*idx_flat.shape[:-1], n // 16, 16)
    t = np.swapaxes(t, -1, -2)                    # [..., 16, n//16]
    return np.tile(t, (1,) * (t.ndim - 2) + (8, 1)).astype(np.int16)


def _pack_pt(a_flat):
    """[n] -> [128, n//128]: slot i -> [i%128, i//128]."""
    n = a_flat.shape[-1]
    t = a_flat.reshape(*a_flat.shape[:-1], n // 128, 128)
    return np.swapaxes(t, -1, -2).copy()


def preprocess(cfg, x, edge_row, edge_col, edge_val):
    """Shard x, build per-core gather/selector metadata. Returns (per_core, TPB)."""
    er = np.asarray(edge_row).astype(np.int64)
    ec = np.asarray(edge_col).astype(np.int64)
    ev = np.asarray(edge_val).astype(np.float32)

    owner = er // cfg.NSH
    row_loc = er % cfg.NSH
    blk = row_loc // P                            # block within core
    rel = (row_loc % P).astype(np.float32)
    c_owner = ec // cfg.NSH
    c_loc = ec % cfg.NSH
    half = (c_loc >= cfg.HALF).astype(np.int64)
    tbl_idx = (c_owner * cfg.HALF + c_loc - half * cfg.HALF).astype(np.int64)

    cores = []
    max_cnt = 0
    for r in range(NCORES):
        m = owner == r
        cores.append((blk[m], half[m], tbl_idx[m], ev[m], rel[m]))
        key = blk[m] * 2 + half[m]
        cnt = np.bincount(key, minlength=cfg.NBLK * 2)
        max_cnt = max(max_cnt, int(cnt.max()))
    TPB = max(1, -(-max_cnt // P))                # tiles per (block, half)
    NIDX = cfg.SBB * TPB * P                      # gather-call size

    per_core = []
    for r in range(NCORES):
        b, h, ti, v, rl = cores[r]
        key = b * 2 + h
        # secondary sort by table index: monotone gather addresses within each
        # (block, half) group give far better HBM row locality
        order = np.argsort(key * 32768 + ti, kind="stable")
        b, h, ti, v, rl = b[order], h[order], ti[order], v[order], rl[order]
        cnt = np.bincount(key[order], minlength=cfg.NBLK * 2)
        # slot of edge j within its (b,h) group
        within = np.arange(len(b)) - np.repeat(
            np.concatenate([[0], np.cumsum(cnt)[:-1]]), cnt)
        # flat slot in [h, s, NIDX] layout
        s = b // cfg.SBB
        bb = b % cfg.SBB
        slot = bb * TPB * P + within
        idx_fill = -1 if cfg.negpad else 0
        idx_arr = np.full((2, cfg.NSB, NIDX), idx_fill, np.int16)
        val_arr = np.zeros((2, cfg.NSB, NIDX), np.float32)
        row_arr = np.zeros((2, cfg.NSB, NIDX), np.float32)
        idx_arr[h, s, slot] = ti.astype(np.int16)
        val_arr[h, s, slot] = v
        row_arr[h, s, slot] = rl

        xT = np.zeros((cfg.DIN, cfg.RPAD), np.float32)
        xT[:, : cfg.NSH] = np.asarray(x[r * cfg.NSH:(r + 1) * cfg.NSH]).T
        NPT = cfg.SBB * TPB
        val_p = _pack_pt(val_arr).astype(cfg.tnp)         # [2,NSB,128,NPT]
        row_p = _pack_pt(row_arr).astype(cfg.tnp)
        if cfg.sel_pair:
            NPT_P = NPT + NPT % 2
            pad = ((0, 0), (0, 0), (0, 0), (0, NPT_P - NPT))
            val_p = np.pad(val_p, pad).reshape(2, cfg.NSB, P, NPT_P // 2, 2)
            row_p = np.pad(row_p, pad).reshape(2, cfg.NSB, P, NPT_P // 2, 2)
        per_core.append(dict(
            xT=np.ascontiguousarray(xT),
            idx=_pack_idx(idx_arr),                       # [2,NSB,128,NIDX//16]
            val=val_p,
            rowrel=row_p,
        ))
    return per_core, TPB


# -------------------------------------------------------------- device side


def build_program(cfg, TPB, dt_val, no_cc=False, reps=1):
    H, C, DIN, L = cfg.H, cfg.C, cfg.DIN, cfg.L
    NIDX = cfg.SBB * TPB * P
    NPT = cfg.SBB * TPB
    NL = L + 2                                    # number of tables/layers
    # final layer zero-padded C->H so gather elems stay 256 B (fp16 x 128)
    fdims = [H] * (L + 1) + [H]
    fdt = cfg.tdt
    f32 = mybir.dt.float32
    relu = mybir.ActivationFunctionType.Relu
    fcopy = mybir.ActivationFunctionType.Copy

    nc = bacc.Bacc("TRN2", target_bir_lowering=False, debug=False,
                   num_devices=NCORES, num_swdge_queues=cfg.nq)

    xT_d = nc.dram_tensor("xT", [DIN, cfg.RPAD], f32, kind="ExternalInput")
    w1_d = nc.dram_tensor("w1", [DIN, H], f32, kind="ExternalInput")
    b1_d = nc.dram_tensor("b1", [1, H], f32, kind="ExternalInput")
    wm_d = nc.dram_tensor("wm", [L, H, H], f32, kind="ExternalInput")
    bm_d = nc.dram_tensor("bm", [L, 1, H], f32, kind="ExternalInput")
    w2_d = nc.dram_tensor("w2", [H, H], f32, kind="ExternalInput")
    b2_d = nc.dram_tensor("b2", [1, H], f32, kind="ExternalInput")
    NPT_P = NPT + NPT % 2                         # padded even for pair layout
    NPH = NPT_P // 2
    iota_shape = [P, NPH, P, 2] if cfg.sel_pair else [P, P, NPT]
    iota_d = nc.dram_tensor("iota_rep", iota_shape, fdt, kind="ExternalInput")
    idx_d = nc.dram_tensor("idx", [2, cfg.NSB, P, NIDX // 16], mybir.dt.int16,
                           kind="ExternalInput")
    meta_shape = [2, cfg.NSB, P, NPH, 2] if cfg.sel_pair \
        else [2, cfg.NSB, P, NPT]
    val_d = nc.dram_tensor("val", meta_shape, fdt, kind="ExternalInput")
    row_d = nc.dram_tensor("rowrel", meta_shape, fdt, kind="ExternalInput")
    out_d = nc.dram_tensor("out", [cfg.NSH, C], f32, kind="ExternalOutput")

    ag_in = [[nc.dram_tensor(f"ag{l}_{h}", [cfg.HALF, fdims[l]], fdt)
              for h in (0, 1)] for l in range(NL)]
    tables = [[nc.dram_tensor(f"table{l}_{h}", [cfg.TBL, fdims[l]], fdt,
                              addr_space="Shared")
               for h in (0, 1)] for l in range(NL)]

    with tile.TileContext(nc) as tc:
        import contextlib
        with contextlib.ExitStack() as ctx:
            const = ctx.enter_context(tc.tile_pool(name="const", bufs=1))
            htp = ctx.enter_context(tc.tile_pool(name="ht", bufs=1))
            accp = ctx.enter_context(tc.tile_pool(name="acc", bufs=1))
            psum = ctx.enter_context(tc.tile_pool(name="psum", bufs=5, space="PSUM"))
            psumd = ctx.enter_context(tc.tile_pool(name="psumd", bufs=3, space="PSUM"))
            meta = ctx.enter_context(tc.tile_pool(name="meta", bufs=3))
            gpool = ctx.enter_context(tc.tile_pool(name="g", bufs=2))
            selp = ctx.enter_context(tc.tile_pool(name="sel", bufs=2))
            yp = ctx.enter_context(tc.tile_pool(name="y", bufs=4))

            nc.gpsimd.load_library(library_config.mlp)

            # ---- constants
            nkt = DIN // P                       # k-tiles for layer-1 dense
            w1_sb = [const.tile([P, H], f32, name=f"w1sb{k}")
                     for k in range(nkt)]
            for k in range(nkt):
                nc.sync.dma_start(w1_sb[k][:], w1_d[k * P:(k + 1) * P, :])
            b1_sb = const.tile([1, H], f32)
            nc.sync.dma_start(b1_sb[:], b1_d[:])
            wm_sb = [const.tile([P, H], f32, name=f"wmsb{i}")
                     for i in range(L)]
            bm_sb = [const.tile([1, H], f32, name=f"bmsb{i}")
                     for i in range(L)]
            for i in range(L):
                nc.sync.dma_start(wm_sb[i][:], wm_d[i])
                nc.sync.dma_start(bm_sb[i][:], bm_d[i])
            w2_sb = const.tile([P, H], f32)
            nc.sync.dma_start(w2_sb[:], w2_d[:])
            b2_sb = const.tile([1, H], f32)
            nc.sync.dma_start(b2_sb[:], b2_d[:])
            iota_sb = const.tile(iota_shape, fdt)
            nc.sync.dma_start(iota_sb[:], iota_d[:])
            ones_sb = const.tile([1, P], f32)
            nc.vector.memset(ones_sb[:], 1.0)
            if cfg.mm_dummy:
                dum_sb = const.tile([P, P], fdt)
                nc.vector.memset(dum_sb[:], 0.0)

            def evict(out_ap, in_ap):
                if cfg.act_ops:
                    nc.scalar.activation(out=out_ap, in_=in_ap, func=fcopy)
                else:
                    nc.vector.tensor_copy(out=out_ap, in_=in_ap)


            ht = htp.tile([P, cfg.RPAD], f32)
            acc = accp.tile([P, cfg.RPAD], f32)
            if cfg.negpad:
                for _i in range(2):   # wipe both rotating g buffers once
                    g0 = gpool.tile([P, NPT, H], fdt, tag="g")
                    nc.vector.memset(g0[:], 0.0)

            dense_w = [w1_sb] + [[wm_sb[i]] for i in range(L)] + [[w2_sb]]
            dense_b = [b1_sb] + [bm_sb[i] for i in range(L)] + [b2_sb]

            def dense_block(l, m, lhs_tiles):
                """y[block m] = lhsT.T @ W + b -> ag_in[l] halves (f16)."""
                F = fdims[l]
                ps = psumd.tile([P, F], f32, tag="psd")
                for k, lt in enumerate(lhs_tiles):
                    nc.tensor.matmul(
                        out=ps[:], lhsT=lt[:, m * P:(m + 1) * P],
                        rhs=dense_w[l][k][:], start=(k == 0), stop=False)
                nc.tensor.matmul(out=ps[:], lhsT=ones_sb[:], rhs=dense_b[l][:],
                                 start=False, stop=True)
                ysb = yp.tile([P, F], fdt, tag="ysb")
                evict(ysb[:], ps[:])
                r0 = m * P
                r1 = min(cfg.NSH, r0 + P)
                for h in (0, 1):
                    lo = max(r0, h * cfg.HALF)
                    hi = min(r1, (h + 1) * cfg.HALF)
                    if lo < hi:
                        nc.scalar.dma_start(
                            out=ag_in[l][h][lo - h * cfg.HALF:hi - h * cfg.HALF, :],
                            in_=ysb[lo - r0:hi - r0, :])

            def allgather(l, h):
                if no_cc:
                    # timeline-profiling stand-in: local copy, same deps
                    nc.sync.dma_start(out=tables[l][h][0:cfg.HALF, :],
                                      in_=ag_in[l][h][:])
                    return
                nc.gpsimd.collective_compute(
                    "AllGather", mybir.AluOpType.bypass,
                    replica_groups=[list(range(NCORES))],
                    ins=[ag_in[l][h][:]], outs=[tables[l][h][:]])

            def spmm_pass(l, h, fuse_dense):
                """One half-pass of spmm over tables[l][h].

                h==0: park per-block partial sums in acc.
                h==1: combine with acc, apply layer update, write ht/out;
                      fuse_dense: emit next layer's dense per finished block;
                      returns after issuing AG(l+1) chunks at s==NSB//2-1 / end.
                """
                final = l == NL - 1
                F = fdims[l]
                for s in range(cfg.NSB):
                    it = meta.tile([P, NIDX // 16], mybir.dt.int16, tag="it")
                    nc.sync.dma_start(it[:], idx_d[h, s])
                    mshape = [P, NPH, 2] if cfg.sel_pair else [P, NPT]
                    vt = meta.tile(mshape, fdt, tag="vt")
                    nc.sync.dma_start(vt[:], val_d[h, s])
                    rt = meta.tile(mshape, fdt, tag="rt")
                    nc.sync.dma_start(rt[:], row_d[h, s])
                    gt = gpool.tile([P, NPT, F], fdt, tag="g")
                    if cfg.gath_lite:
                        nc.gpsimd.dma_gather(
                            gt[:, 0:7, :], tables[l][h][:], it[:, 0:56],
                            7 * P, 7 * P, F, single_packet=True)
                    elif cfg.big_gather:
                        nc.gpsimd.dma_gather(
                            gt[:], tables[l][h][:], it[:], NIDX, NIDX, F,
                            single_packet=False, queue_num=s % cfg.nq)
                    elif cfg.negpad:
                        # block-aligned chunks so pad (idx=-1) is trailing
                        # per call and its descriptors are skipped
                        ci = 0
                        for bb in range(cfg.SBB):
                            c0 = bb * TPB
                            while c0 < (bb + 1) * TPB:
                                c1 = min((bb + 1) * TPB, c0 + 8)
                                nc.gpsimd.dma_gather(
                                    gt[:, c0:c1, :], tables[l][h][:],
                                    it[:, c0 * 8:c1 * 8],
                                    (c1 - c0) * P, (c1 - c0) * P, F,
                                    single_packet=True,
                                    queue_num=ci % cfg.nq)
                                ci += 1
                                c0 = c1
                    else:
                        CH = cfg.ch
                        for ci, c0 in enumerate(range(0, NPT, CH)):
                            c1 = min(NPT, c0 + CH)
                            nc.gpsimd.dma_gather(
                                gt[:, c0:c1, :], tables[l][h][:],
                                it[:, c0 * 8:c1 * 8],
                                (c1 - c0) * P, (c1 - c0) * P, F,
                                single_packet=True,
                                queue_num=ci % cfg.nq)
                    # batched selector: SEL[e, r, j] = (r == rowrel[e,j]) * val[e,j]
                    # all operands' last dims packed -> 2x fp16 DVE mode
                    if cfg.sel_pair:
                        sel = selp.tile([P, NPH, P, 2], fdt, tag="sel")
                        rt_b = rt[:][:, :, None, :].to_broadcast([P, NPH, P, 2])
                        vt_b = vt[:][:, :, None, :].to_broadcast([P, NPH, P, 2])
                    else:
                        sel = selp.tile([P, P, NPT], fdt, tag="sel")
                        rt_b = rt[:][:, None, :].to_broadcast([P, P, NPT])
                        vt_b = vt[:][:, None, :].to_broadcast([P, P, NPT])
                    if cfg.sel_lite:
                        nc.vector.memset(sel[:, 0, :, :], 0.0)
                    else:
                        nc.vector.tensor_tensor(
                            out=sel[:], in0=iota_sb[:], in1=rt_b,
                            op=mybir.AluOpType.is_equal)
                        nc.vector.tensor_tensor(
                            out=sel[:], in0=sel[:], in1=vt_b,
                            op=mybir.AluOpType.mult)
                    for bb in range(cfg.SBB):
                        b = s * cfg.SBB + bb
                        if final:
                            ps = psum.tile([P, F], f32, tag="ps")
                        else:
                            ps = psum.tile([F, P], f32, tag="ps")
                        for t in range(1 if cfg.mm_lite else TPB):
                            j = bb * TPB + t
                            if cfg.mm_lite:
                                t = TPB - 1  # start+stop in one
                            if cfg.mm_dummy:
                                sl = dum_sb[:]
                            elif cfg.sel_pair:
                                sl = sel[:, j // 2, :, j % 2]
                            else:
                                sl = sel[:, :, j]
                            st = (t == 0) or cfg.mm_lite
                            if final:
                                nc.tensor.matmul(
                                    out=ps[:], lhsT=sl, rhs=gt[:, j, :],
                                    start=st, stop=(t == TPB - 1))
                            else:
                                nc.tensor.matmul(
                                    out=ps[:], lhsT=gt[:, j, :], rhs=sl,
                                    start=st, stop=(t == TPB - 1))
                        co = b * P
                        if h == 0:
                            evict(acc[:, co:co + P], ps[:])
                            continue
                        # ---- h == 1: combine + layer update (all DVE;
                        # dt is pre-folded into wm/bm so no scale needed)
                        if final:
                            osb = yp.tile([P, P], f32, tag="osb")
                            nc.vector.tensor_tensor(
                                out=osb[:], in0=ps[:], in1=acc[:, co:co + P],
                                op=mybir.AluOpType.add)
                            r0 = b * P
                            r1 = min(cfg.NSH, r0 + P)
                            if r0 < r1:
                                nc.scalar.dma_start(out=out_d[r0:r1, :],
                                                    in_=osb[: r1 - r0, 0:C])
                        elif l == 0:
                            tmp = yp.tile([P, P], f32, tag="tmp")
                            nc.vector.tensor_tensor(
                                out=tmp[:], in0=ps[:], in1=acc[:, co:co + P],
                                op=mybir.AluOpType.add)
                            nc.vector.tensor_scalar(
                                out=ht[:, co:co + P], in0=tmp[:],
                                scalar1=0.0, scalar2=None,
                                op0=mybir.AluOpType.max)
                        else:
                            tmp = yp.tile([P, P], f32, tag="tmp")
                            nc.vector.tensor_tensor(
                                out=tmp[:], in0=ps[:], in1=acc[:, co:co + P],
                                op=mybir.AluOpType.add)
                            # ht += relu(tmp), fused: (tmp max 0) add ht
                            nc.vector.scalar_tensor_tensor(
                                out=ht[:, co:co + P], in0=tmp[:], scalar=0.0,
                                in1=ht[:, co:co + P],
                                op0=mybir.AluOpType.max,
                                op1=mybir.AluOpType.add)
                        if fuse_dense:
                            dense_block(l + 1, b, [ht])
                    if h == 1 and fuse_dense and s == cfg.ag_point:
                        allgather(l + 1, 0)
                if h == 1 and fuse_dense:
                    if cfg.ag_point >= cfg.NSB:
                        allgather(l + 1, 0)
                    allgather(l + 1, 1)

            # ---- layer 0 dense (from xT in DRAM); reps>1 = timing
            # amplification only (identical recompute, same output)
            for _rep in range(reps):
                with tc.tile_pool(name="xt", bufs=1) as xtp:
                    xt_sb = xtp.tile([P, nkt * cfg.RPAD], f32)
                    for k in range(nkt):
                        nc.sync.dma_start(
                            xt_sb[:, k * cfg.RPAD:(k + 1) * cfg.RPAD],
                            xT_d[k * P:(k + 1) * P, :])
                    xt_tiles = [xt_sb[:, k * cfg.RPAD:(k + 1) * cfg.RPAD]
                                for k in range(nkt)]
                    for m in range(cfg.NBLK):
                        dense_block(0, m, xt_tiles)
                        if m == cfg.NBLK // 2 - 1 and cfg.ag_point < cfg.NSB:
                            allgather(0, 0)
                    if cfg.ag_point >= cfg.NSB:
                        allgather(0, 0)
                    allgather(0, 1)
                for l in range(NL):
                    spmm_pass(l, 0, fuse_dense=False)
                    spmm_pass(l, 1, fuse_dense=(l < NL - 1))

    nc.compile()
    return nc


# ------------------------------------------------------------------ driver

_CACHE = {}


def _get_program(cfg, TPB, dt_val):
    key = (cfg.N, cfg.E, cfg.big_gather, cfg.sel_pair, cfg.act_ops,
           cfg.mm_dummy, cfg.ag_point, cfg.nq, cfg.ch, cfg.negpad,
           TPB, float(dt_val))
    if key not in _CACHE:
        _CACHE[key] = build_program(cfg, TPB, dt_val)
    return _CACHE[key]


def make_in_maps(cfg, inputs, per_core, TPB):
    dt_val = float(np.asarray(inputs["time_step"]))
    NPT = cfg.SBB * TPB
    NPH = (NPT + NPT % 2) // 2
    ar = np.arange(P, dtype=cfg.tnp)
    if cfg.sel_pair:
        iota_rep = np.broadcast_to(
            ar[None, None, :, None], (P, NPH, P, 2)).copy()
    else:
        iota_rep = np.broadcast_to(ar[None, :, None], (P, P, NPT)).copy()
    shared = dict(
        w1=np.asarray(inputs["w1"], np.float32),
        b1=np.asarray(inputs["b1"], np.float32).reshape(1, cfg.H),
        wm=np.asarray(inputs["wm"], np.float32) * dt_val,
        bm=np.asarray(inputs["bm"], np.float32).reshape(cfg.L, 1, cfg.H)
        * dt_val,
        w2=np.pad(np.asarray(inputs["w2"], np.float32),
                  ((0, 0), (0, cfg.H - cfg.C))),
        b2=np.pad(np.asarray(inputs["b2"], np.float32).reshape(1, cfg.C),
                  ((0, 0), (0, cfg.H - cfg.C))),
        iota_rep=iota_rep,
    )
    return [{**shared, **pc} for pc in per_core]


def prepare(cfg, inputs):
    """Preprocess inputs and build (cached) program. Returns (nc, in_maps)."""
    x = np.asarray(inputs["x"], np.float32)
    per_core, TPB = preprocess(cfg, x, inputs["edge_row"], inputs["edge_col"],
                               inputs["edge_val"])
    dt_val = float(np.asarray(inputs["time_step"]))
    nc = _get_program(cfg, TPB, dt_val)
    return nc, make_in_maps(cfg, inputs, per_core, TPB)


def run(cfg, inputs):
    nc, in_maps = prepare(cfg, inputs)
    res = run_bass_kernel_spmd(nc, in_maps, list(range(NCORES)))
    out = np.concatenate([res.results[r]["out"] for r in range(NCORES)], axis=0)
    return out.astype(np.float32)


def kernel(**inputs) -> np.ndarray:
    return run(CFG_FULL, inputs)


# ---------------------------------------------------- timing helper (test use)


def make_timed_runner(nc, in_maps):
    """Build a reusable jitted runner (no donation, device-resident operands).

    Mirrors bass2jax.run_bass_via_pjrt's multi-core path but keeps the jitted
    callable and device arrays so repeated calls measure dispatch+exec only.
    Returns (call_fn, out_unpack_fn).
    """
    import jax
    from jax.sharding import Mesh, PartitionSpec
    from jax.experimental.shard_map import shard_map
    from concourse import bass2jax
    from concourse.bass2jax import _bass_exec_p, partition_id_tensor

    bass2jax.install_neuronx_cc_hook()
    n_cores = len(in_maps)
    partition_name = nc.partition_id_tensor.name if nc.partition_id_tensor else None
    in_names, out_names, out_avals, zero_outs = [], [], [], []
    for alloc in nc.m.functions[0].allocations:
        if not isinstance(alloc, mybir.MemoryLocationSet):
            continue
        name = alloc.memorylocations[0].name
        if alloc.kind == "ExternalInput":
            if name != partition_name:
                in_names.append(name)
        elif alloc.kind == "ExternalOutput":
            out_names.append(name)
            out_avals.append(jax.core.ShapedArray(
                tuple(alloc.tensor_shape), mybir.dt.np(alloc.dtype)))
            zero_outs.append(np.zeros(tuple(alloc.tensor_shape),
                                      mybir.dt.np(alloc.dtype)))
    n_params = len(in_names)
    all_names = in_names + out_names
    if partition_name is not None:
        all_names.append(partition_name)

    def _body(*args):
        operands = list(args)
        if partition_name is not None:
            operands.append(partition_id_tensor())
        return tuple(_bass_exec_p.bind(
            *operands,
            out_avals=tuple(out_avals),
            in_names=tuple(all_names),
            out_names=tuple(out_names),
            lowering_input_output_aliases=(),
            sim_require_finite=True,
            sim_require_nnan=True,
            nc=nc,
        ))

    devices = jax.devices()[:n_cores]
    mesh = Mesh(np.asarray(devices), ("core",))
    spec_in = (PartitionSpec("core"),) * (n_params + len(out_names))
    spec_out = (PartitionSpec("core"),) * len(out_names)
    fn = jax.jit(shard_map(_body, mesh=mesh, in_specs=spec_in,
                           out_specs=spec_out, check_rep=False),
                 keep_unused=True)

    sharding = jax.sharding.NamedSharding(mesh, PartitionSpec("core"))
    dev_args = []
    for i, name in enumerate(in_names):
        cat = np.concatenate([np.asarray(m[name]) for m in in_maps], axis=0)
        dev_args.append(jax.device_put(cat, sharding))
    for z in zero_outs:
        cat = np.zeros((n_cores * z.shape[0], *z.shape[1:]), z.dtype)
        dev_args.append(jax.device_put(cat, sharding))

    def call():
        outs = fn(*dev_args)
        jax.block_until_ready(outs)
        return outs

    def unpack(outs):
        return [
            {name: np.asarray(outs[i]).reshape(n_cores, *out_avals[i].shape)[c]
             for i, name in enumerate(out_names)}
            for c in range(n_cores)
        ]

    return call, unpack
